# revision 1
# baseline (speedup 1.0000x reference)
"""2-layer GCN (GCNConv x2) on trn2 x8 NeuronCores.

Strategy: dst-shard nodes across 8 cores. Per-node norm factorization
(dinv = 1/sqrt(deg+1)) turns the GCN edge norm into pre/post row scales, so
propagation is a pure segment-sum:  h[d] = dinv_d * (sum_{s in N(d)} y[s] + y[d]).
Segment-sum runs on the TensorEngine: edges sorted by (src-chunk, dst-window)
are processed in 128-edge tiles; a one-hot selection matrix S (DVE is_equal vs
iota) maps each edge lane to its 64-wide window slot, and PSUM accumulates
S^T @ gathered_rows.  Feature rows (bf16, 256B) are fetched with dma_gather
(int16 indices, 4 table chunks) from an AllGather-replicated table.  Layer 2
propagates scalars via the same machinery on a replicated z-table.
"""

import sys

sys.path.insert(0, "/opt/trn_rl_repo")

import numpy as np

from concourse import bacc, bass, mybir, tile
from concourse import bass_utils
from concourse.library_config import mlp
from concourse.masks import make_identity

F32 = mybir.dt.float32
BF16 = mybir.dt.bfloat16
I16 = mybir.dt.int16
AF = mybir.ActivationFunctionType
ALU = mybir.AluOpType

# problem sizes (hardcoded per spec)
N = 100000
E = 1600000
D = 256
H = 128
NC = 8
NPC = N // NC                  # 12500 nodes per core
NTILE = (NPC + 127) // 128     # 98 node tiles per core
NPAD = NTILE * 128             # 12544
WIN = 64                       # dst window width (one-hot columns)
NW = NPAD // WIN               # 196 windows per core
TBLROWS = NC * NPAD            # 100352 replicated-table rows
CH = 4                         # int16 table chunks
TB = 8                         # tiles per gather batch
DMA_SCRATCH = 16384


def _host_prep(edge_index):
    """Index-only host prep: edge partitioning/sorting and gather-row ids."""
    global CROWS
    CROWS = TBLROWS // CH
    src = np.asarray(edge_index[0], dtype=np.int64)
    dst = np.asarray(edge_index[1], dtype=np.int64)

    deg = np.bincount(dst, minlength=N).astype(np.float32) + 1.0  # incl self loop

    core = dst // NPC
    dl = dst - core * NPC
    w = dl // WIN

    # table row for src node: core cs, local ls=t*128+p -> cs*NPAD + p*NTILE + t
    cs = src // NPC
    ls = src - cs * NPC
    row = cs * NPAD + (ls % 128) * NTILE + (ls // 128)
    chunk = row // CROWS
    row16 = (row % CROWS).astype(np.int64)

    cnt = np.zeros((NC, CH, NW), dtype=np.int64)
    np.add.at(cnt, (core, chunk, w), 1)
    Twc = np.maximum(1, (cnt.max(axis=0) + 127) // 128)  # [CH, NW] tiles per group

    # global tile order: chunk-major, then window
    Ttot = int(Twc.sum())
    tstart = np.zeros((CH, NW), dtype=np.int64)
    flat = Twc.reshape(-1)
    starts = np.concatenate([[0], np.cumsum(flat)[:-1]])
    tstart = starts.reshape(CH, NW)
    seg = [(int(Twc[:c].sum()), int(Twc[:c + 1].sum())) for c in range(CH)]

    tile_w = np.empty(Ttot, dtype=np.int64)
    tile_c = np.empty(Ttot, dtype=np.int64)
    for c in range(CH):
        for wi in range(NW):
            t0 = tstart[c, wi]
            tile_w[t0:t0 + Twc[c, wi]] = wi
            tile_c[t0:t0 + Twc[c, wi]] = c

    idx16 = np.zeros((NC, 128, Ttot * 8), dtype=np.int16)   # pad -> row 0
    dstl = np.full((NC, 128, Ttot), -1.0, dtype=np.float32)  # pad -> -1

    gkey = chunk * NW + w
    for c in range(NC):
        msk = core == c
        kc = gkey[msk]
        o = np.argsort(kc, kind="stable")
        kc = kc[o]
        rowc = row16[msk][o]
        dlwc = (dl[msk][o] % WIN).astype(np.float32)
        grp_start = np.searchsorted(kc, np.arange(CH * NW))
        pos = np.arange(len(kc)) - grp_start[kc]
        slot = tstart.reshape(-1)[kc] * 128 + pos
        p = slot % 128
        t = slot // 128
        dstl[c, p, t] = dlwc
        # dma_gather idx layout: logical i at [i%16 + 16k, i//16] for k in 0..7
        col = t * 8 + p // 16
        pr = p % 16
        for k in range(8):
            idx16[c, pr + 16 * k, col] = rowc

    degs = np.ones((NC, 128, NTILE), dtype=np.float32)
    degr = deg.reshape(NC, NPC)
    for c in range(NC):
        dc = np.ones(NPAD, dtype=np.float32)
        dc[:NPC] = degr[c]
        degs[c] = dc.reshape(NTILE, 128).T

    return dict(Twc=Twc, tstart=tstart, seg=seg, Ttot=Ttot, tile_w=tile_w,
                tile_c=tile_c, idx16=idx16, dstl=dstl, degs=degs)


def _build_nc(meta):
    Twc, tstart, seg, Ttot = meta["Twc"], meta["tstart"], meta["seg"], meta["Ttot"]
    tile_w, tile_c = meta["tile_w"], meta["tile_c"]
    CROWS = TBLROWS // CH

    nc = bacc.Bacc("TRN2", target_bir_lowering=False, debug=False, num_devices=NC,
                   dynamic_dma_scratch_size=DMA_SCRATCH)

    xc_d = nc.dram_tensor("xc", [NPAD, D], F32, kind="ExternalInput")
    deg_d = nc.dram_tensor("deg", [128, NTILE], F32, kind="ExternalInput")
    w1_d = nc.dram_tensor("w1", [2, 128, H], F32, kind="ExternalInput")
    w2rep_d = nc.dram_tensor("w2rep", [128, H], F32, kind="ExternalInput")
    b1rep_d = nc.dram_tensor("b1rep", [128, H], F32, kind="ExternalInput")
    b2rep_d = nc.dram_tensor("b2rep", [128, 1], F32, kind="ExternalInput")
    iota_d = nc.dram_tensor("iota", [128, WIN], F32, kind="ExternalInput")
    idx_d = nc.dram_tensor("idx16", [128, Ttot * 8], I16, kind="ExternalInput")
    dstl_d = nc.dram_tensor("dstl", [128, Ttot], F32, kind="ExternalInput")
    out_d = nc.dram_tensor("out", [128, NTILE], F32, kind="ExternalOutput")

    yb_d = nc.dram_tensor("y_bounce", [128, NTILE, H], BF16)
    yfull_d = nc.dram_tensor("y_full", [TBLROWS, H], BF16)
    zb_d = nc.dram_tensor("z_bounce", [128, NTILE, H], BF16)
    zfull_d = nc.dram_tensor("z_full", [TBLROWS, H], BF16)

    rg = [list(range(NC))]

    with tile.TileContext(nc) as tc:
        with (
            tc.tile_pool(name="persist", bufs=1) as pp,
            tc.tile_pool(name="xload", bufs=3) as xp,
            tc.tile_pool(name="small", bufs=3) as sp,
            tc.tile_pool(name="gbuf", bufs=2) as gp,
            tc.tile_pool(name="ibuf", bufs=3) as ip,
            tc.tile_pool(name="sgen", bufs=2) as sgp,
            tc.tile_pool(name="epi", bufs=3) as ep,
            tc.tile_pool(name="pacc", bufs=2, space="PSUM") as pap,
            tc.tile_pool(name="ptmp", bufs=2, space="PSUM") as ptp,
        ):
            y_sb = pp.tile([128, NTILE * H], F32, tag="y")
            tbl_sb = pp.tile([128, NTILE * H], BF16, tag="tbl")  # y/z2 staging
            acc_sb = pp.tile([128, NTILE * H], F32, tag="acc")
            acc2_sb = pp.tile([128, NTILE], F32, tag="acc2")
            deg_sb = pp.tile([128, NTILE], F32, tag="deg")
            dinv_sb = pp.tile([128, NTILE], F32, tag="dinv")
            w1_sb = pp.tile([128, 2 * H], F32, tag="w1")
            w2_sb = pp.tile([128, H], F32, tag="w2")
            b1_sb = pp.tile([128, H], F32, tag="b1")
            b2_sb = pp.tile([128, 1], F32, tag="b2")
            iota_sb = pp.tile([128, WIN], F32, tag="iota")
            dstl_sb = pp.tile([128, Ttot], F32, tag="dstl")
            z2_sb = pp.tile([128, NTILE], F32, tag="z2")
            out_sb = pp.tile([128, NTILE], F32, tag="out")
            ident_sb = pp.tile([128, 128], F32, tag="ident")

            nc.sync.dma_start(deg_sb[:], deg_d[:, :])
            nc.sync.dma_start(w1_sb[:, 0:H], w1_d[0, :, :])
            nc.sync.dma_start(w1_sb[:, H:2 * H], w1_d[1, :, :])
            nc.sync.dma_start(w2_sb[:], w2rep_d[:, :])
            nc.sync.dma_start(b1_sb[:], b1rep_d[:, :])
            nc.sync.dma_start(b2_sb[:], b2rep_d[:, :])
            nc.sync.dma_start(iota_sb[:], iota_d[:, :])
            nc.sync.dma_start(dstl_sb[:], dstl_d[:, :])
            make_identity(nc, ident_sb[:])
            nc.scalar.activation(dinv_sb[:], deg_sb[:], AF.Sqrt)
            nc.vector.reciprocal(dinv_sb[:], dinv_sb[:])

            # ---- phase A: y = dinv * (x @ W1) ----
            for t in range(NTILE):
                x_t = xp.tile([128, D], F32, tag="x")
                nc.sync.dma_start(x_t[:], xc_d[t * 128:(t + 1) * 128, :])
                ym = ptp.tile([128, H], F32, tag="ym")
                for k in range(2):
                    xT = ptp.tile([128, 128], F32, tag="xT")
                    nc.tensor.transpose(
                        out=xT[:], in_=x_t[:, k * 128:(k + 1) * 128],
                        identity=ident_sb[:],
                    )
                    xT_sb = sp.tile([128, 128], F32, tag="xTs")
                    nc.vector.tensor_copy(xT_sb[:], xT[:])
                    nc.tensor.matmul(
                        out=ym[:], lhsT=xT_sb[:], rhs=w1_sb[:, k * H:(k + 1) * H],
                        start=(k == 0), stop=(k == 1),
                    )
                nc.scalar.activation(y_sb[:, t * H:(t + 1) * H], ym[:], AF.Copy,
                                     scale=dinv_sb[:, t:t + 1])
                nc.scalar.activation(tbl_sb[:, t * H:(t + 1) * H], ym[:], AF.Copy,
                                     scale=dinv_sb[:, t:t + 1])

            nc.sync.dma_start(yb_d.ap().rearrange("p t h -> p (t h)"), tbl_sb[:])
            nc.gpsimd.collective_compute(
                "AllGather", ALU.bypass, replica_groups=rg,
                ins=[yb_d.ap().opt()], outs=[yfull_d.ap().opt()],
            )
            nc.gpsimd.load_library(mlp)

            def propagate(table_d, pass2):
                width = 1 if pass2 else H
                atag = "a2" if pass2 else "a1"
                acc = None
                pend = {}  # nt -> psum tile awaiting flush
                t = 0
                while t < Ttot:
                    c = int(tile_c[t])
                    b1_ = min(t + TB, seg[c][1])  # batch within chunk segment
                    nb = b1_ - t
                    idxb = ip.tile([128, TB * 8], I16, tag="idx")
                    nc.sync.dma_start(idxb[:, :nb * 8],
                                      idx_d[:, t * 8:(t + nb) * 8])
                    g = gp.tile([128, TB, H], BF16, tag="g")
                    nc.gpsimd.dma_gather(
                        out_ap=g[:, :nb, :],
                        in_ap=table_d[c * CROWS:(c + 1) * CROWS, :],
                        idxs_ap=idxb[:, :nb * 8],
                        num_idxs=nb * 128, num_idxs_reg=nb * 128,
                        elem_size=H,
                    )
                    S_b = sgp.tile([128, TB, WIN], BF16, tag="S")
                    nc.vector.tensor_tensor(
                        out=S_b[:, :nb, :],
                        in0=dstl_sb[:, t:t + nb].rearrange("p n -> p n ()")
                            .to_broadcast([128, nb, WIN]),
                        in1=iota_sb[:].rearrange("p w -> p () w")
                            .to_broadcast([128, nb, WIN]),
                        op=ALU.is_equal,
                    )
                    for j in range(nb):
                        tt = t + j
                        wi = int(tile_w[tt])
                        ci = int(tile_c[tt])
                        nt, half = divmod(wi, 2)
                        first = tt == int(tstart[ci, wi])
                        last = tt == int(tstart[ci, wi]) + int(Twc[ci, wi]) - 1
                        if half == 0 and first:
                            acc = pap.tile([128, width], F32, tag=atag)
                            pend[nt] = acc
                        rhs = g[:, j, :] if not pass2 else g[:, j, 0:1]
                        nc.tensor.matmul(
                            out=pend[nt][half * WIN:(half + 1) * WIN, :],
                            lhsT=S_b[:, j, :], rhs=rhs,
                            start=first, stop=last,
                        )
                        if half == 1 and last:
                            a = pend.pop(nt)
                            if pass2:
                                dst_ap = acc2_sb[:, nt:nt + 1]
                            else:
                                dst_ap = acc_sb[:, nt * H:(nt + 1) * H]
                            if ci == 0:
                                nc.vector.tensor_copy(dst_ap, a[:])
                            else:
                                nc.vector.tensor_tensor(
                                    out=dst_ap, in0=dst_ap, in1=a[:], op=ALU.add)
                    t = b1_

            propagate(yfull_d, pass2=False)

            # ---- pass-1 epilogue per node tile ----
            for t in range(NTILE):
                u = ep.tile([128, H], F32, tag="u")
                nc.vector.tensor_tensor(
                    out=u[:], in0=acc_sb[:, t * H:(t + 1) * H],
                    in1=y_sb[:, t * H:(t + 1) * H], op=ALU.add)
                v = ep.tile([128, H], F32, tag="v")
                nc.scalar.activation(v[:], u[:], AF.Copy,
                                     scale=dinv_sb[:, t:t + 1])
                nc.vector.tensor_tensor(out=v[:], in0=v[:], in1=b1_sb[:],
                                        op=ALU.add)
                hrel = ep.tile([128, H], F32, tag="h")
                nc.scalar.activation(hrel[:], v[:], AF.Relu)
                hw = ep.tile([128, H], F32, tag="hw")
                nc.vector.tensor_tensor(out=hw[:], in0=hrel[:], in1=w2_sb[:],
                                        op=ALU.mult)
                z = ep.tile([128, 1], F32, tag="z")
                nc.vector.reduce_sum(z[:], hw[:], axis=mybir.AxisListType.X)
                nc.vector.tensor_scalar(
                    out=z2_sb[:, t:t + 1], in0=z[:],
                    scalar1=dinv_sb[:, t:t + 1], scalar2=None, op0=ALU.mult)
                # replicate z2 into bf16 table rows
                nc.vector.tensor_copy(
                    tbl_sb[:, t * H:(t + 1) * H],
                    z2_sb[:, t:t + 1].to_broadcast([128, H]))

            nc.sync.dma_start(zb_d.ap().rearrange("p t h -> p (t h)"), tbl_sb[:])
            nc.gpsimd.collective_compute(
                "AllGather", ALU.bypass, replica_groups=rg,
                ins=[zb_d.ap().opt()], outs=[zfull_d.ap().opt()],
            )

            propagate(zfull_d, pass2=True)

            # ---- pass-2 epilogue ----
            for t in range(NTILE):
                u = ep.tile([128, 1], F32, tag="u2")
                nc.vector.tensor_tensor(
                    out=u[:], in0=acc2_sb[:, t:t + 1], in1=z2_sb[:, t:t + 1],
                    op=ALU.add)
                nc.vector.tensor_scalar(
                    out=out_sb[:, t:t + 1], in0=u[:],
                    scalar1=dinv_sb[:, t:t + 1], scalar2=b2_sb[:],
                    op0=ALU.mult, op1=ALU.add)

            nc.sync.dma_start(out_d[:, :], out_sb[:])

    nc.compile()
    return nc


def kernel(x, edge_index, W1, b1, W2, b2):
    x = np.asarray(x, dtype=np.float32)
    edge_index = np.asarray(edge_index)
    W1 = np.asarray(W1, dtype=np.float32)
    b1 = np.asarray(b1, dtype=np.float32)
    W2 = np.asarray(W2, dtype=np.float32)
    b2 = np.asarray(b2, dtype=np.float32)

    meta = _host_prep(edge_index)
    nc = _build_nc(meta)

    iota = np.broadcast_to(np.arange(WIN, dtype=np.float32), (128, WIN)).copy()
    w1_in = W1.reshape(2, 128, H).copy()
    w2rep = np.broadcast_to(W2[:, 0], (128, H)).copy().astype(np.float32)
    b1rep = np.broadcast_to(b1, (128, H)).copy().astype(np.float32)
    b2rep = np.full((128, 1), float(b2[0]), dtype=np.float32)

    in_maps = []
    for c in range(NC):
        xc = np.zeros((NPAD, D), dtype=np.float32)
        xc[:NPC] = x[c * NPC:(c + 1) * NPC]
        in_maps.append({
            "xc": xc,
            "deg": meta["degs"][c],
            "w1": w1_in,
            "w2rep": w2rep,
            "b1rep": b1rep,
            "b2rep": b2rep,
            "iota": iota,
            "idx16": meta["idx16"][c],
            "dstl": meta["dstl"][c],
        })

    import time as _time
    _t0 = _time.time()
    res = bass_utils.run_bass_kernel_spmd(nc, in_maps, core_ids=list(range(NC)))
    kernel._exec_wall_ns = int((_time.time() - _t0) * 1e9)
    kernel._last = res

    out = np.empty(N, dtype=np.float32)
    for c in range(NC):
        o = res.results[c]["out"]
        out[c * NPC:(c + 1) * NPC] = o.T.reshape(-1)[:NPC]
    return out



# revision 3
# speedup vs baseline: 1.0181x; 1.0181x over previous
"""2-layer GCN (GCNConv x2) on trn2 x8 NeuronCores.

Strategy: dst-shard nodes across 8 cores.  Per-node norm factorization
(dinv = 1/sqrt(deg+1)) turns the GCN edge norm into pre/post row scales, so
propagation is a pure segment-sum over src rows (self-loops are folded in as
ordinary edges).  Each core computes y = dinv*(x@W1) for its node shard from
a host-pre-transposed bf16 x (196 matmuls total), AllGathers the bf16 y
table, then per 128-dst-node tile dma_gathers every dst's neighbor rows into
[128, K, H] (4 table chunks to satisfy the int16 index range; padding slots
point at an all-zero table row) and segment-sums with a single strided
tensor_reduce on the vector engine.  Layer 2 replicates the per-node scalar
z = dinv*(relu(h)@W2) across a 128-wide bf16 row and reuses the exact same
gather indices (the z table mirrors the y table layout).  Keeping every
engine queue well under ~5k instructions avoids the superlinear NEFF-load
cliff that dominated wall time; input bytes are minimized (bf16 x, compact
[16, X] int16 indices replicated to 128 partitions on-device).
"""

import sys

sys.path.insert(0, "/opt/trn_rl_repo")

import numpy as np
import ml_dtypes

from concourse import bacc, bass, mybir, tile
from concourse import bass_utils
from concourse.library_config import mlp

F32 = mybir.dt.float32
BF16 = mybir.dt.bfloat16
I16 = mybir.dt.int16
AF = mybir.ActivationFunctionType
ALU = mybir.AluOpType
AX = mybir.AxisListType

# problem sizes (hardcoded per spec)
N = 100000
E = 1600000
D = 256
H = 128
NC = 8
NPC = N // NC                  # 12500 nodes per core
NTILE = (NPC + 127) // 128     # 98 node tiles per core
NPAD = NTILE * 128             # 12544
TBLROWS = NC * NPAD            # 100352 replicated-table rows
CH = 4                         # int16 table chunks
CROWS = TBLROWS // CH          # 25088 rows per chunk (< 32768)
ZROW = NPAD - 1                # 12543: all-zero row id within every chunk


def _host_prep(edge_index):
    """Index-only host prep: per-(dst-tile, chunk) gather indices + degrees."""
    src = np.asarray(edge_index[0]).astype(np.int64, copy=False)
    dst = np.asarray(edge_index[1]).astype(np.int64, copy=False)
    loop = np.arange(N, dtype=np.int64)
    src = np.concatenate([src, loop])
    dst = np.concatenate([dst, loop])

    deg = np.bincount(dst, minlength=N).astype(np.float32)  # incl self loop

    # y/z table row of each src node: core cs, local ls=t*128+p -> row p*NTILE+t
    cs = src // NPC
    ls = src - cs * NPC
    row = cs * NPAD + (ls % 128) * NTILE + (ls // 128)
    chunk = row // CROWS
    r16 = (row - chunk * CROWS).astype(np.int16)

    core = dst // NPC
    dl = dst - core * NPC
    tl = dl // 128
    p = dl - tl * 128

    # group edges by (core, tile, chunk, partition); j = rank within group
    key = ((core * NTILE + tl) * CH + chunk) * 128 + p
    nkey = NC * NTILE * CH * 128
    order = np.argsort(key, kind="stable")
    ks = key[order]
    grp_start = np.searchsorted(ks, np.arange(nkey))
    j = np.arange(len(ks), dtype=np.int64) - grp_start[ks]
    cnt = np.bincount(key, minlength=nkey)
    # SPMD: one program for all cores -> K = max over cores & partitions
    K = cnt.reshape(NC, NTILE, CH, 128).max(axis=(0, 3)).astype(np.int64)  # [NTILE, CH]
    blocks = 128 * K
    off = np.zeros(NTILE * CH, dtype=np.int64)
    off[1:] = np.cumsum(blocks.reshape(-1))[:-1]
    TOT = int(blocks.sum())

    pos = off[(tl * CH + chunk)[order]] + j * 128 + p[order]
    idxflat = np.full((NC, TOT), ZROW, dtype=np.int16)
    idxflat[core[order], pos] = r16[order]
    idx16 = np.ascontiguousarray(
        idxflat.reshape(NC, TOT // 16, 16).transpose(0, 2, 1))  # [NC, 16, TOT/16]

    degs = np.ones((NC, 128, NTILE), dtype=np.float32)
    degr = deg.reshape(NC, NPC)
    for c in range(NC):
        dc = np.ones(NPAD, dtype=np.float32)
        dc[:NPC] = degr[c]
        degs[c] = dc.reshape(NTILE, 128).T

    return dict(K=K, TOT=TOT, idx16=idx16, degs=degs)


def _build_nc(meta):
    K, TOT = meta["K"], meta["TOT"]
    KTOT = K.sum(axis=1)                  # [NTILE] total gathered slots per dst
    KMAX = int(KTOT.max())

    nc = bacc.Bacc("TRN2", target_bir_lowering=False, debug=False, num_devices=NC,
                   dynamic_dma_scratch_size=16384)

    xt_d = nc.dram_tensor("xt", [2, 128, NPAD], BF16, kind="ExternalInput")
    deg_d = nc.dram_tensor("deg", [128, NTILE], F32, kind="ExternalInput")
    w1_d = nc.dram_tensor("w1", [2, 128, H], BF16, kind="ExternalInput")
    b1_d = nc.dram_tensor("b1rep", [128, H], F32, kind="ExternalInput")
    w2_d = nc.dram_tensor("w2rep", [128, H], F32, kind="ExternalInput")
    b2_d = nc.dram_tensor("b2rep", [128, 1], F32, kind="ExternalInput")
    mask_d = nc.dram_tensor("padmask", [128, 1], F32, kind="ExternalInput")
    idx_d = nc.dram_tensor("idx16", [16, TOT // 16], I16, kind="ExternalInput")
    out_d = nc.dram_tensor("out", [128, NTILE], F32, kind="ExternalOutput")

    yb_d = nc.dram_tensor("y_bounce", [128, NTILE, H], BF16)
    yfull_d = nc.dram_tensor("y_full", [TBLROWS, H], BF16)
    zb_d = nc.dram_tensor("z_bounce", [128, NTILE, H], BF16)
    zfull_d = nc.dram_tensor("z_full", [TBLROWS, H], BF16)

    rg = [list(range(NC))]

    with tile.TileContext(nc) as tc:
        with tc.tile_pool(name="persist", bufs=1) as pp:
            w1_sb = pp.tile([128, 2 * H], BF16, tag="w1")
            b1_sb = pp.tile([128, H], F32, tag="b1")
            w2_sb = pp.tile([128, H], F32, tag="w2")
            b2_sb = pp.tile([128, 1], F32, tag="b2")
            mask_sb = pp.tile([128, 1], F32, tag="mask")
            deg_sb = pp.tile([128, NTILE], F32, tag="deg")
            dinv_sb = pp.tile([128, NTILE], F32, tag="dinv")
            idx_sb = pp.tile([128, TOT // 16], I16, tag="idx")
            z2_sb = pp.tile([128, NTILE], F32, tag="z2")
            out_sb = pp.tile([128, NTILE], F32, tag="out")

            nc.sync.dma_start(deg_sb[:], deg_d[:, :])
            nc.sync.dma_start(w1_sb[:, 0:H], w1_d[0, :, :])
            nc.sync.dma_start(w1_sb[:, H:2 * H], w1_d[1, :, :])
            nc.sync.dma_start(b1_sb[:], b1_d[:, :])
            nc.sync.dma_start(w2_sb[:], w2_d[:, :])
            nc.sync.dma_start(b2_sb[:], b2_d[:, :])
            nc.sync.dma_start(mask_sb[:], mask_d[:, :])
            for k in range(8):
                nc.sync.dma_start(idx_sb[16 * k:16 * (k + 1), :], idx_d[:, :])
            nc.scalar.activation(dinv_sb[:], deg_sb[:], AF.Sqrt)
            nc.vector.reciprocal(dinv_sb[:], dinv_sb[:])

            # ---- phase A: y = dinv * (x @ W1), straight to bf16 table ----
            with (
                tc.tile_pool(name="xload", bufs=1) as xp,
                tc.tile_pool(name="ytmp", bufs=3) as yp,
                tc.tile_pool(name="pacc", bufs=2, space="PSUM") as pap,
            ):
                xt_sb = xp.tile([128, 2 * NPAD], BF16, tag="xt")
                nc.sync.dma_start(xt_sb[:, 0:NPAD], xt_d[0, :, :])
                nc.sync.dma_start(xt_sb[:, NPAD:2 * NPAD], xt_d[1, :, :])
                for t in range(NTILE):
                    ym = pap.tile([128, H], F32, tag="ym")
                    for k in range(2):
                        nc.tensor.matmul(
                            out=ym[:],
                            lhsT=xt_sb[:, k * NPAD + t * 128:k * NPAD + (t + 1) * 128],
                            rhs=w1_sb[:, k * H:(k + 1) * H],
                            start=(k == 0), stop=(k == 1),
                        )
                    y_t = yp.tile([128, H], BF16, tag="yt")
                    nc.scalar.activation(y_t[:], ym[:], AF.Copy,
                                         scale=dinv_sb[:, t:t + 1])
                    nc.sync.dma_start(yb_d[:, t, :], y_t[:])

            nc.gpsimd.collective_compute(
                "AllGather", ALU.bypass, replica_groups=rg,
                ins=[yb_d.ap().opt()], outs=[yfull_d.ap().opt()],
            )
            nc.gpsimd.load_library(mlp)

            KSUB = 8  # <=1024 idxs per gather: descs must fit the DMA scratch

            def gather_tile(gp, table_d, t, ioffs):
                g = gp.tile([128, KMAX, H], BF16, tag="g")
                coloff = 0
                for ch in range(CH):
                    Kc = int(K[t, ch])
                    ioff = int(ioffs[t * CH + ch])
                    for k0 in range(0, Kc, KSUB):
                        kk = min(KSUB, Kc - k0)
                        ni = 128 * kk
                        io = ioff + 128 * k0
                        nc.gpsimd.dma_gather(
                            out_ap=g[:, coloff + k0:coloff + k0 + kk, :],
                            in_ap=table_d[ch * CROWS:(ch + 1) * CROWS, :],
                            idxs_ap=idx_sb[:, io // 16:(io + ni) // 16],
                            num_idxs=ni, num_idxs_reg=ni, elem_size=H,
                        )
                    coloff += Kc
                return g

            blocks = (128 * K).reshape(-1)
            ioffs = np.zeros(NTILE * CH, dtype=np.int64)
            ioffs[1:] = np.cumsum(blocks)[:-1]

            # ---- pass 1: h = relu(dinv*(segsum y)+b1); z = dinv*(h@W2) ----
            with (
                tc.tile_pool(name="gbuf", bufs=3) as gp,
                tc.tile_pool(name="work", bufs=3) as wp,
            ):
                for t in range(NTILE):
                    g = gather_tile(gp, yfull_d, t, ioffs)
                    kt = int(KTOT[t])
                    acc = wp.tile([128, H], F32, tag="acc")
                    nc.vector.tensor_reduce(
                        out=acc[:], in_=g[:, 0:kt, :].rearrange("p k h -> p h k"),
                        axis=AX.X, op=ALU.add)
                    h = wp.tile([128, H], F32, tag="h")
                    nc.vector.tensor_scalar(
                        out=h[:], in0=acc[:], scalar1=dinv_sb[:, t:t + 1],
                        scalar2=None, op0=ALU.mult)
                    nc.vector.tensor_tensor(out=h[:], in0=h[:], in1=b1_sb[:],
                                            op=ALU.add)
                    nc.scalar.activation(h[:], h[:], AF.Relu)
                    hw = wp.tile([128, H], F32, tag="hw")
                    nc.vector.tensor_tensor(out=hw[:], in0=h[:], in1=w2_sb[:],
                                            op=ALU.mult)
                    u = wp.tile([128, 1], F32, tag="u")
                    nc.vector.reduce_sum(u[:], hw[:], axis=AX.X)
                    nc.vector.tensor_scalar(
                        out=z2_sb[:, t:t + 1], in0=u[:],
                        scalar1=dinv_sb[:, t:t + 1], scalar2=None, op0=ALU.mult)
                    if t == NTILE - 1:
                        # zero the 44 pad slots so the z table's ZROW stays 0
                        nc.vector.tensor_scalar(
                            out=z2_sb[:, t:t + 1], in0=z2_sb[:, t:t + 1],
                            scalar1=mask_sb[:], scalar2=None, op0=ALU.mult)
                    zr = wp.tile([128, H], BF16, tag="zr")
                    nc.vector.tensor_copy(zr[:], z2_sb[:, t:t + 1]
                                          .to_broadcast([128, H]))
                    nc.sync.dma_start(zb_d[:, t, :], zr[:])

            nc.gpsimd.collective_compute(
                "AllGather", ALU.bypass, replica_groups=rg,
                ins=[zb_d.ap().opt()], outs=[zfull_d.ap().opt()],
            )

            # ---- pass 2: out = dinv*(segsum z) + b2 ----
            with (
                tc.tile_pool(name="gbuf2", bufs=3) as gp2,
                tc.tile_pool(name="work2", bufs=3) as wp2,
            ):
                for t in range(NTILE):
                    g = gather_tile(gp2, zfull_d, t, ioffs)
                    kt = int(KTOT[t])
                    a2 = wp2.tile([128, 1], F32, tag="a2")
                    nc.vector.tensor_reduce(
                        out=a2[:], in_=g[:, 0:kt, 0:1].rearrange("p k h -> p h k"),
                        axis=AX.X, op=ALU.add)
                    nc.vector.tensor_scalar(
                        out=out_sb[:, t:t + 1], in0=a2[:],
                        scalar1=dinv_sb[:, t:t + 1], scalar2=b2_sb[:],
                        op0=ALU.mult, op1=ALU.add)

            nc.sync.dma_start(out_d[:, :], out_sb[:])

    nc.compile()
    return nc


def kernel(x, edge_index, W1, b1, W2, b2):
    x = np.asarray(x, dtype=np.float32)
    W1 = np.asarray(W1, dtype=np.float32)
    b1 = np.asarray(b1, dtype=np.float32)
    W2 = np.asarray(W2, dtype=np.float32)
    b2 = np.asarray(b2, dtype=np.float32)

    meta = _host_prep(edge_index)
    nc = _build_nc(meta)

    BF = ml_dtypes.bfloat16
    xt = np.zeros((NC, 256, NPAD), dtype=BF)
    xr = x.reshape(NC, NPC, D)
    for c in range(NC):
        xt[c, :, :NPC] = xr[c].T
    xt = xt.reshape(NC, 2, 128, NPAD)

    w1_in = W1.astype(BF).reshape(2, 128, H)
    b1rep = np.broadcast_to(b1, (128, H)).astype(np.float32)
    w2rep = np.broadcast_to(W2[:, 0], (128, H)).astype(np.float32)
    b2rep = np.full((128, 1), float(b2[0]), dtype=np.float32)
    padmask = (np.arange(128) < (NPC - (NTILE - 1) * 128)).astype(
        np.float32).reshape(128, 1)

    in_maps = []
    for c in range(NC):
        in_maps.append({
            "xt": xt[c],
            "deg": meta["degs"][c],
            "w1": w1_in,
            "b1rep": b1rep,
            "w2rep": w2rep,
            "b2rep": b2rep,
            "padmask": padmask,
            "idx16": meta["idx16"][c],
        })

    import time as _time
    _t0 = _time.time()
    res = bass_utils.run_bass_kernel_spmd(nc, in_maps, core_ids=list(range(NC)))
    kernel._exec_wall_ns = int((_time.time() - _t0) * 1e9)
    kernel._last = res

    out = np.empty(N, dtype=np.float32)
    for c in range(NC):
        o = res.results[c]["out"]
        out[c * NPC:(c + 1) * NPC] = o.T.reshape(-1)[:NPC]
    return out


# revision 5
# speedup vs baseline: 3.8860x; 3.8170x over previous
"""2-layer GCN (GCNConv x2) on trn2 x8 NeuronCores.

Strategy: dst-shard nodes across 8 cores.  Per-node norm factorization
(dinv = 1/sqrt(deg+1)) turns the GCN edge norm into pre/post row scales, so
propagation is a pure segment-sum over src rows (self-loops are folded in as
ordinary edges).  Each core computes y = dinv*(x@W1) for its node shard from
a host-pre-transposed bf16 x (196 matmuls total), AllGathers the bf16 y
table, then per 128-dst-node tile dma_gathers every dst's neighbor rows into
[128, K, H] (4 table chunks to satisfy the int16 index range; padding slots
point at an all-zero table row) and segment-sums with a single strided
tensor_reduce on the vector engine.  Layer 2 replicates the per-node scalar
z = dinv*(relu(h)@W2) across a 128-wide bf16 row and reuses the exact same
gather indices (the z table mirrors the y table layout).  Keeping every
engine queue well under ~5k instructions avoids the superlinear NEFF-load
cliff that dominated wall time; input bytes are minimized (bf16 x, compact
[16, X] int16 indices replicated to 128 partitions on-device).
"""

import sys

sys.path.insert(0, "/opt/trn_rl_repo")

import numpy as np
import ml_dtypes

from concourse import bacc, bass, mybir, tile
from concourse import bass_utils
from concourse.library_config import mlp

F32 = mybir.dt.float32
BF16 = mybir.dt.bfloat16
I16 = mybir.dt.int16
AF = mybir.ActivationFunctionType
ALU = mybir.AluOpType
AX = mybir.AxisListType

# problem sizes (hardcoded per spec)
N = 100000
E = 1600000
D = 256
H = 128
NC = 8
NPC = N // NC                  # 12500 nodes per core
NTILE = (NPC + 127) // 128     # 98 node tiles per core
NPAD = NTILE * 128             # 12544
TBLROWS = NC * NPAD            # 100352 replicated-table rows
CH = 4                         # int16 table chunks
CROWS = TBLROWS // CH          # 25088 rows per chunk (< 32768)
ZROW = NPAD - 1                # 12543: all-zero row id within every chunk


def _host_prep(edge_index):
    """Index-only host prep: per-(dst-tile, chunk) gather indices + degrees."""
    src = np.asarray(edge_index[0]).astype(np.int64, copy=False)
    dst = np.asarray(edge_index[1]).astype(np.int64, copy=False)
    loop = np.arange(N, dtype=np.int64)
    src = np.concatenate([src, loop])
    dst = np.concatenate([dst, loop])

    deg = np.bincount(dst, minlength=N).astype(np.float32)  # incl self loop

    # y/z table row of each src node: core cs, local ls=t*128+p -> row p*NTILE+t
    cs = src // NPC
    ls = src - cs * NPC
    row = cs * NPAD + (ls % 128) * NTILE + (ls // 128)
    chunk = row // CROWS
    r16 = (row - chunk * CROWS).astype(np.int16)

    core = dst // NPC
    dl = dst - core * NPC
    tl = dl // 128
    p = dl - tl * 128

    # group edges by (core, tile, chunk, partition); j = rank within group
    key = ((core * NTILE + tl) * CH + chunk) * 128 + p
    nkey = NC * NTILE * CH * 128
    order = np.argsort(key, kind="stable")
    ks = key[order]
    grp_start = np.searchsorted(ks, np.arange(nkey))
    j = np.arange(len(ks), dtype=np.int64) - grp_start[ks]
    cnt = np.bincount(key, minlength=nkey)
    # SPMD: one program for all cores -> K = max over cores & partitions
    K = cnt.reshape(NC, NTILE, CH, 128).max(axis=(0, 3)).astype(np.int64)  # [NTILE, CH]
    blocks = 128 * K
    off = np.zeros(NTILE * CH, dtype=np.int64)
    off[1:] = np.cumsum(blocks.reshape(-1))[:-1]
    TOT = int(blocks.sum())

    pos = off[(tl * CH + chunk)[order]] + j * 128 + p[order]
    idxflat = np.full((NC, TOT), ZROW, dtype=np.int16)
    idxflat[core[order], pos] = r16[order]
    idx16 = np.ascontiguousarray(
        idxflat.reshape(NC, TOT // 16, 16).transpose(0, 2, 1))  # [NC, 16, TOT/16]

    degs = np.ones((NC, 128, NTILE), dtype=np.float32)
    degr = deg.reshape(NC, NPC)
    for c in range(NC):
        dc = np.ones(NPAD, dtype=np.float32)
        dc[:NPC] = degr[c]
        degs[c] = dc.reshape(NTILE, 128).T

    return dict(K=K, TOT=TOT, idx16=idx16, degs=degs)


def _build_nc(meta):
    K, TOT = meta["K"], meta["TOT"]
    KTOT = K.sum(axis=1)                  # [NTILE] total gathered slots per dst
    KMAX = int(KTOT.max())

    nc = bacc.Bacc("TRN2", target_bir_lowering=False, debug=False, num_devices=NC,
                   dynamic_dma_scratch_size=16384)

    xt_d = nc.dram_tensor("xt", [2, 128, NPAD], BF16, kind="ExternalInput")
    deg_d = nc.dram_tensor("deg", [128, NTILE], F32, kind="ExternalInput")
    w1_d = nc.dram_tensor("w1", [2, 128, H], BF16, kind="ExternalInput")
    b1_d = nc.dram_tensor("b1rep", [128, H], F32, kind="ExternalInput")
    w2_d = nc.dram_tensor("w2rep", [128, H], F32, kind="ExternalInput")
    b2_d = nc.dram_tensor("b2rep", [128, 1], F32, kind="ExternalInput")
    mask_d = nc.dram_tensor("padmask", [128, 1], F32, kind="ExternalInput")
    idx_d = nc.dram_tensor("idx16", [16, TOT // 16], I16, kind="ExternalInput")
    out_d = nc.dram_tensor("out", [128, NTILE], F32, kind="ExternalOutput")

    yb_d = nc.dram_tensor("y_bounce", [128, NTILE, H], BF16)
    yfull_d = nc.dram_tensor("y_full", [TBLROWS, H], BF16)
    zb_d = nc.dram_tensor("z_bounce", [128, NTILE, H], BF16)
    zfull_d = nc.dram_tensor("z_full", [TBLROWS, H], BF16)

    rg = [list(range(NC))]

    with tile.TileContext(nc) as tc:
        with tc.tile_pool(name="persist", bufs=1) as pp:
            w1_sb = pp.tile([128, 2 * H], BF16, tag="w1")
            b1_sb = pp.tile([128, H], F32, tag="b1")
            w2_sb = pp.tile([128, H], F32, tag="w2")
            b2_sb = pp.tile([128, 1], F32, tag="b2")
            mask_sb = pp.tile([128, 1], F32, tag="mask")
            deg_sb = pp.tile([128, NTILE], F32, tag="deg")
            dinv_sb = pp.tile([128, NTILE], F32, tag="dinv")
            idx_sb = pp.tile([128, TOT // 16], I16, tag="idx")
            z2_sb = pp.tile([128, NTILE], F32, tag="z2")
            out_sb = pp.tile([128, NTILE], F32, tag="out")

            nc.sync.dma_start(deg_sb[:], deg_d[:, :])
            nc.sync.dma_start(w1_sb[:, 0:H], w1_d[0, :, :])
            nc.sync.dma_start(w1_sb[:, H:2 * H], w1_d[1, :, :])
            nc.sync.dma_start(b1_sb[:], b1_d[:, :])
            nc.sync.dma_start(w2_sb[:], w2_d[:, :])
            nc.sync.dma_start(b2_sb[:], b2_d[:, :])
            nc.sync.dma_start(mask_sb[:], mask_d[:, :])
            for k in range(8):
                nc.sync.dma_start(idx_sb[16 * k:16 * (k + 1), :], idx_d[:, :])
            nc.scalar.activation(dinv_sb[:], deg_sb[:], AF.Sqrt)
            nc.vector.reciprocal(dinv_sb[:], dinv_sb[:])

            # ---- phase A: y = dinv * (x @ W1), straight to bf16 table ----
            with (
                tc.tile_pool(name="xload", bufs=1) as xp,
                tc.tile_pool(name="ytmp", bufs=3) as yp,
                tc.tile_pool(name="pacc", bufs=2, space="PSUM") as pap,
            ):
                xt_sb = xp.tile([128, 2 * NPAD], BF16, tag="xt")
                nc.sync.dma_start(xt_sb[:, 0:NPAD], xt_d[0, :, :])
                nc.sync.dma_start(xt_sb[:, NPAD:2 * NPAD], xt_d[1, :, :])
                for t in range(NTILE):
                    ym = pap.tile([128, H], F32, tag="ym")
                    for k in range(2):
                        nc.tensor.matmul(
                            out=ym[:],
                            lhsT=xt_sb[:, k * NPAD + t * 128:k * NPAD + (t + 1) * 128],
                            rhs=w1_sb[:, k * H:(k + 1) * H],
                            start=(k == 0), stop=(k == 1),
                        )
                    y_t = yp.tile([128, H], BF16, tag="yt")
                    nc.scalar.activation(y_t[:], ym[:], AF.Copy,
                                         scale=dinv_sb[:, t:t + 1])
                    nc.sync.dma_start(yb_d[:, t, :], y_t[:])

            nc.gpsimd.collective_compute(
                "AllGather", ALU.bypass, replica_groups=rg,
                ins=[yb_d.ap().opt()], outs=[yfull_d.ap().opt()],
            )
            nc.gpsimd.load_library(mlp)

            KSUB = 8  # <=1024 idxs per gather: hard ucode limit
            # hoist num_idxs_reg registers: one RegisterMove per distinct
            # count instead of one per gather call
            ni_regs = {}

            def ni_reg(ni):
                if ni not in ni_regs:
                    ni_regs[ni] = nc.gpsimd.to_reg(ni)
                return ni_regs[ni]

            def gather_tile(gp, table_d, t, ioffs):
                g = gp.tile([128, KMAX, H], BF16, tag="g")
                coloff = 0
                for ch in range(CH):
                    Kc = int(K[t, ch])
                    ioff = int(ioffs[t * CH + ch])
                    for k0 in range(0, Kc, KSUB):
                        kk = min(KSUB, Kc - k0)
                        ni = 128 * kk
                        io = ioff + 128 * k0
                        nc.gpsimd.dma_gather(
                            out_ap=g[:, coloff + k0:coloff + k0 + kk, :],
                            in_ap=table_d[ch * CROWS:(ch + 1) * CROWS, :],
                            idxs_ap=idx_sb[:, io // 16:(io + ni) // 16],
                            num_idxs=ni, num_idxs_reg=ni_reg(ni), elem_size=H,
                        )
                    coloff += Kc
                return g

            blocks = (128 * K).reshape(-1)
            ioffs = np.zeros(NTILE * CH, dtype=np.int64)
            ioffs[1:] = np.cumsum(blocks)[:-1]

            # ---- pass 1: h = relu(dinv*(segsum y)+b1); z = dinv*(h@W2) ----
            with (
                tc.tile_pool(name="gbuf", bufs=3) as gp,
                tc.tile_pool(name="work", bufs=3) as wp,
            ):
                for t in range(NTILE):
                    g = gather_tile(gp, yfull_d, t, ioffs)
                    kt = int(KTOT[t])
                    acc = wp.tile([128, H], F32, tag="acc")
                    nc.vector.tensor_reduce(
                        out=acc[:], in_=g[:, 0:kt, :].rearrange("p k h -> p h k"),
                        axis=AX.X, op=ALU.add)
                    h = wp.tile([128, H], F32, tag="h")
                    nc.vector.tensor_scalar(
                        out=h[:], in0=acc[:], scalar1=dinv_sb[:, t:t + 1],
                        scalar2=None, op0=ALU.mult)
                    nc.vector.tensor_tensor(out=h[:], in0=h[:], in1=b1_sb[:],
                                            op=ALU.add)
                    nc.scalar.activation(h[:], h[:], AF.Relu)
                    hw = wp.tile([128, H], F32, tag="hw")
                    nc.vector.tensor_tensor(out=hw[:], in0=h[:], in1=w2_sb[:],
                                            op=ALU.mult)
                    u = wp.tile([128, 1], F32, tag="u")
                    nc.vector.reduce_sum(u[:], hw[:], axis=AX.X)
                    nc.vector.tensor_scalar(
                        out=z2_sb[:, t:t + 1], in0=u[:],
                        scalar1=dinv_sb[:, t:t + 1], scalar2=None, op0=ALU.mult)
                    if t == NTILE - 1:
                        # zero the 44 pad slots so the z table's ZROW stays 0
                        nc.vector.tensor_scalar(
                            out=z2_sb[:, t:t + 1], in0=z2_sb[:, t:t + 1],
                            scalar1=mask_sb[:], scalar2=None, op0=ALU.mult)
                    zr = wp.tile([128, H], BF16, tag="zr")
                    nc.vector.tensor_copy(zr[:], z2_sb[:, t:t + 1]
                                          .to_broadcast([128, H]))
                    nc.sync.dma_start(zb_d[:, t, :], zr[:])

            nc.gpsimd.collective_compute(
                "AllGather", ALU.bypass, replica_groups=rg,
                ins=[zb_d.ap().opt()], outs=[zfull_d.ap().opt()],
            )

            # ---- pass 2: out = dinv*(segsum z) + b2 ----
            with (
                tc.tile_pool(name="gbuf2", bufs=3) as gp2,
                tc.tile_pool(name="work2", bufs=3) as wp2,
            ):
                for t in range(NTILE):
                    g = gather_tile(gp2, zfull_d, t, ioffs)
                    kt = int(KTOT[t])
                    a2 = wp2.tile([128, 1], F32, tag="a2")
                    nc.vector.tensor_reduce(
                        out=a2[:], in_=g[:, 0:kt, 0:1].rearrange("p k h -> p h k"),
                        axis=AX.X, op=ALU.add)
                    nc.vector.tensor_scalar(
                        out=out_sb[:, t:t + 1], in0=a2[:],
                        scalar1=dinv_sb[:, t:t + 1], scalar2=b2_sb[:],
                        op0=ALU.mult, op1=ALU.add)

            nc.sync.dma_start(out_d[:, :], out_sb[:])

    nc.compile()
    return nc


def kernel(x, edge_index, W1, b1, W2, b2):
    x = np.asarray(x, dtype=np.float32)
    W1 = np.asarray(W1, dtype=np.float32)
    b1 = np.asarray(b1, dtype=np.float32)
    W2 = np.asarray(W2, dtype=np.float32)
    b2 = np.asarray(b2, dtype=np.float32)

    meta = _host_prep(edge_index)
    nc = _build_nc(meta)

    BF = ml_dtypes.bfloat16
    xt = np.zeros((NC, 256, NPAD), dtype=BF)
    xr = x.reshape(NC, NPC, D)
    for c in range(NC):
        xt[c, :, :NPC] = xr[c].T
    xt = xt.reshape(NC, 2, 128, NPAD)

    w1_in = W1.astype(BF).reshape(2, 128, H)
    b1rep = np.broadcast_to(b1, (128, H)).astype(np.float32)
    w2rep = np.broadcast_to(W2[:, 0], (128, H)).astype(np.float32)
    b2rep = np.full((128, 1), float(b2[0]), dtype=np.float32)
    padmask = (np.arange(128) < (NPC - (NTILE - 1) * 128)).astype(
        np.float32).reshape(128, 1)

    in_maps = []
    for c in range(NC):
        in_maps.append({
            "xt": xt[c],
            "deg": meta["degs"][c],
            "w1": w1_in,
            "b1rep": b1rep,
            "w2rep": w2rep,
            "b2rep": b2rep,
            "padmask": padmask,
            "idx16": meta["idx16"][c],
        })

    import time as _time
    _t0 = _time.time()
    res = bass_utils.run_bass_kernel_spmd(nc, in_maps, core_ids=list(range(NC)))
    kernel._exec_wall_ns = int((_time.time() - _t0) * 1e9)
    kernel._last = res

    out = np.empty(N, dtype=np.float32)
    for c in range(NC):
        o = res.results[c]["out"]
        out[c * NPC:(c + 1) * NPC] = o.T.reshape(-1)[:NPC]
    return out
